# revision 13
# baseline (speedup 1.0000x reference)
"""Trainium2 Bass kernel for nn_MinDistanceLoss (min-image distance loss).

Math (per atom n in structure b, with M = cell[b], G = M^T M):
    delta_n = x_n - x_tilde_n
    off*_n  = argmin_{off in {-1,0,1}^3} ||M (delta_n + off)||^2
    d_n     = delta_n + off*_n
    center_b = mean_{n in b} d_n
    loss    = mean |d_n - center_b|        (mean over all atoms and 3 comps)

Key optimization: for every atom that satisfies the *certificate*

    CERT_MARGIN * lambda_max_bound(G_b) * |delta_n|^2 < lambda_min_bound(G_b)

(Gershgorin bounds: lambda_max <= max_i sum_j |G_ij|,
 lambda_min >= min_i (G_ii - sum_{j!=i} |G_ij|); CERT_MARGIN = 4.04 > 4),
off* = 0 is provably the unique argmin: ||M off|| >= 2.01 ||M delta|| for
any off != 0 implies ||M(delta+off)|| >= 1.005 ||M delta||, a margin that
also dominates fp32 rounding in the reference computation. For the
physical regime this model targets (cells ~ 4*I Angstrom, |delta| ~ 0.05
fractional) the certificate holds for every atom with >2x margin.

The device computes, per core shard (struct-aligned, planar delta layout):
delta (GPSIMD), per-structure segment sums -> centers -> rt = center-delta
(a hop-free VectorE chain), and the |rt| partition sums via the ScalarE Abs
activation's accum_out. The certificate itself (|delta_n|^2 < thr_b with
thr_b from `cell`'s Gershgorin bounds) is evaluated on the host - it is a
validity guard, not part of the loss dataflow. If any atom fails it, the
host falls back to an exact numpy evaluation of the full 27-image argmin.

Sharding: data-parallel over structures. Core c gets structures
[c*2048, (c+1)*2048) == atoms [c*131072, (c+1)*131072). The only
cross-atom reduction (per-structure scatter-mean) is shard-local; the
final mean is assembled on the host from per-partition partial sums.
"""

import numpy as np
from contextlib import ExitStack

# ---- problem constants (hardcoded per the fixed input spec) ----
B = 16384            # structures
A = 64               # atoms per structure
N = B * A            # total atoms
NCORES = 8
BS = B // NCORES     # structures per core     = 2048
NS = N // NCORES     # atoms per core          = 131072
P = 128              # SBUF partitions
SPP = BS // P        # structures per partition = 16
FC = NS * 3 // P     # f32 elems per partition  = 3072
NCH = 8              # pipeline chunks
SCH = SPP // NCH     # structures per partition per chunk = 2
CW = FC // NCH       # columns per chunk = 384
BUFS = 8
CERT_MARGIN = 4.04

_cache = {}


def _build_nc():
    import concourse.bacc as bacc
    import concourse.tile as tile
    import concourse.mybir as mybir

    f32 = mybir.dt.float32
    Alu = mybir.AluOpType
    Act = mybir.ActivationFunctionType

    nc = bacc.Bacc("TRN2", target_bir_lowering=False, debug=False,
                   num_devices=NCORES)
    xin_d = nc.dram_tensor("xin", [P, 2 * FC], f32, kind="ExternalInput").ap()
    out_d = nc.dram_tensor("out", [P, NCH], f32, kind="ExternalOutput").ap()

    with tile.TileContext(nc) as tc:
        with ExitStack() as ctx:
            singles = ctx.enter_context(tc.tile_pool(name="singles", bufs=1))
            io = ctx.enter_context(tc.tile_pool(name="io", bufs=BUFS))
            work = ctx.enter_context(tc.tile_pool(name="work", bufs=BUFS))
            outs = singles.tile([P, NCH], f32)

            for ch in range(NCH):
                xb = io.tile([P, 2 * CW], f32, tag="xb")
                nc.sync.dma_start(
                    out=xb[:], in_=xin_d[:, ch * 2 * CW:(ch + 1) * 2 * CW])
                xc = xb[:, :CW]
                xtc = xb[:, CW:]
                # planar delta: read [s,a,c]-interleaved, write [s,c,a].
                # Iterate in INPUT order (contiguous 4B reads) and let the
                # writes be strided: the GPSIMD >8B-stride cliff is on the
                # read-side address generator.
                d = work.tile([P, CW], f32, tag="d")
                d_pl = d[:].rearrange("p (s c a) -> p s c a", c=3, a=A)
                with tc.high_priority():
                    nc.gpsimd.tensor_sub(
                        d[:].rearrange("p (s c a) -> p s a c", c=3, a=A),
                        xc.rearrange("p (s a c) -> p s a c", a=A, c=3),
                        xtc.rearrange("p (s a c) -> p s a c", a=A, c=3))
                # hop-free VectorE chain: segment sums -> centers -> rt
                segsum = work.tile([P, SCH, 3], f32, tag="segsum")
                nc.vector.tensor_reduce(out=segsum[:], in_=d_pl,
                                        axis=mybir.AxisListType.X, op=Alu.add)
                csc = work.tile([P, SCH, 3], f32, tag="csc")
                nc.vector.tensor_scalar_mul(csc[:], segsum[:], 1.0 / A)
                rt = work.tile([P, CW], f32, tag="rt")
                rt_pl = rt[:].rearrange("p (s c a) -> p s c a", c=3, a=A)
                nc.vector.tensor_sub(rt_pl,
                                     csc[:].broadcast_to([P, SCH, 3, A]), d_pl)
                # sum |rt| via ScalarE Abs + accumulate
                scrap = work.tile([P, CW], f32, tag="scrap")
                nc.scalar.activation(out=scrap[:], in_=rt[:], func=Act.Abs,
                                     accum_out=outs[:, ch:ch + 1])

            nc.sync.dma_start(out=out_d, in_=outs[:])
    nc.compile()
    return nc


def _get_nc():
    if "nc" not in _cache:
        _cache["nc"] = _build_nc()
    return _cache["nc"]


def _host_thr(cell):
    """Per-structure certificate threshold, from structure-level scalars.

    thr_b = gersh_lo(G_b) / (CERT_MARGIN * rowmax(|G_b|)), G = M^T M.
    Degenerate structures (gersh_lo <= 0, zero rows, NaN) get -inf-ish so
    they always trip the certificate and force the exact fallback.
    """
    c64 = cell.astype(np.float64)
    G = np.einsum("bki,bkj->bij", c64, c64)
    absG = np.abs(G)
    diag = np.diagonal(G, axis1=1, axis2=2)
    rowsum = absG.sum(2)
    gersh_lo = (2.0 * diag - rowsum).min(1)
    rowmax = rowsum.max(1)
    with np.errstate(divide="ignore", invalid="ignore"):
        thr = gersh_lo / (CERT_MARGIN * rowmax)
    bad = ~np.isfinite(thr) | (gersh_lo <= 0) | (rowmax <= 0)
    return np.where(bad, -3.0e38, thr)


# ---------------- exact host fallback (never hit for sane inputs) -------
_OFFSETS = np.array([[a, b, c] for a in (-1, 0, 1) for b in (-1, 0, 1)
                     for c in (-1, 0, 1)], dtype=np.float32)


def _reference_numpy(cell, x, x_tilde, num_atoms):
    cell = np.asarray(cell, np.float32)
    x = np.asarray(x, np.float32)
    x_tilde = np.asarray(x_tilde, np.float32)
    num_atoms = np.asarray(num_atoms)
    n = x.shape[0]
    bounds = np.cumsum(num_atoms)
    batch = np.searchsorted(bounds, np.arange(n), side="right")
    d_all = np.empty_like(x)
    for lo in range(0, n, 131072):
        hi = min(lo + 131072, n)
        cb = cell[batch[lo:hi]]                                   # [m,3,3]
        euc_xt = np.einsum("nij,nj->ni", cb, x_tilde[lo:hi])
        frac = x[lo:hi, None, :] + _OFFSETS[None, :, :]           # [m,27,3]
        euc_x = np.einsum("nij,noj->noi", cb, frac)
        dist = np.linalg.norm(euc_xt[:, None, :] - euc_x, axis=2)
        mi = np.argmin(dist, axis=1)
        d_all[lo:hi] = (x[lo:hi] + _OFFSETS[mi]) - x_tilde[lo:hi]
    sums = np.zeros((num_atoms.shape[0], 3), np.float32)
    np.add.at(sums, batch, d_all)
    center = sums / num_atoms.astype(np.float32)[:, None]
    tot = np.abs(d_all - center[batch]).sum(dtype=np.float64)
    return np.float32(tot / d_all.size)


def _make_in_maps(x, x_tilde):
    in_maps = []
    for c in range(NCORES):
        xr = np.ascontiguousarray(x[c * NS:(c + 1) * NS]).reshape(P, NCH, CW)
        xtr = np.ascontiguousarray(
            x_tilde[c * NS:(c + 1) * NS]).reshape(P, NCH, CW)
        xin = np.concatenate([xr, xtr], axis=2).reshape(P, 2 * FC)
        in_maps.append({"xin": np.ascontiguousarray(xin)})
    return in_maps


def _run_device(x, x_tilde, trace=False):
    from concourse.bass_utils import run_bass_kernel_spmd
    nc = _get_nc()
    return run_bass_kernel_spmd(nc, _make_in_maps(x, x_tilde),
                                core_ids=list(range(NCORES)), trace=trace)


def kernel(cell, x, x_tilde, num_atoms):
    cell = np.asarray(cell)
    x = np.asarray(x)
    x_tilde = np.asarray(x_tilde)
    num_atoms = np.asarray(num_atoms)

    shapes_ok = (cell.shape == (B, 3, 3) and x.shape == (N, 3)
                 and x_tilde.shape == (N, 3) and num_atoms.shape == (B,)
                 and np.all(num_atoms == A))
    if not shapes_ok:
        return _reference_numpy(cell, x, x_tilde, num_atoms)

    res = _run_device(np.asarray(x, np.float32),
                      np.asarray(x_tilde, np.float32))
    # certificate (host, exact l2 form): |delta_n|^2 < thr_b for every atom
    thr = _host_thr(np.asarray(cell, np.float32))          # [B] float64
    delta = x.astype(np.float32) - x_tilde.astype(np.float32)
    d2 = (delta.astype(np.float64) ** 2).sum(1).reshape(B, A)
    if not (np.isfinite(d2).all() and (d2.max(1) < thr).all()):
        # some atom might prefer a non-zero periodic image: exact fallback
        return _reference_numpy(cell, x, x_tilde, num_atoms)
    total = 0.0
    for c in range(NCORES):
        psum = res.results[c]["out"]
        if not np.isfinite(psum).all():
            return _reference_numpy(cell, x, x_tilde, num_atoms)
        total += psum.sum(dtype=np.float64)
    return np.float32(total / (3.0 * N))


# revision 14
# speedup vs baseline: 1.0046x; 1.0046x over previous
"""Trainium2 Bass kernel for nn_MinDistanceLoss (min-image distance loss).

Math (per atom n in structure b, with M = cell[b], G = M^T M):
    delta_n = x_n - x_tilde_n
    off*_n  = argmin_{off in {-1,0,1}^3} ||M (delta_n + off)||^2
    d_n     = delta_n + off*_n
    center_b = mean_{n in b} d_n
    loss    = mean |d_n - center_b|        (mean over all atoms and 3 comps)

Key optimization: for every atom that satisfies the *certificate*

    CERT_MARGIN * lambda_max_bound(G_b) * |delta_n|^2 < lambda_min_bound(G_b)

(Gershgorin bounds: lambda_max <= max_i sum_j |G_ij|,
 lambda_min >= min_i (G_ii - sum_{j!=i} |G_ij|); CERT_MARGIN = 4.04 > 4),
off* = 0 is provably the unique argmin: ||M off|| >= 2.01 ||M delta|| for
any off != 0 implies ||M(delta+off)|| >= 1.005 ||M delta||, a margin that
also dominates fp32 rounding in the reference computation. For the
physical regime this model targets (cells ~ 4*I Angstrom, |delta| ~ 0.05
fractional) the certificate holds for every atom with >2x margin.

The device computes, per core shard (struct-aligned, planar delta layout):
delta (GPSIMD), per-structure segment sums -> centers -> rt = center-delta
(a hop-free VectorE chain), and the |rt| partition sums via the ScalarE Abs
activation's accum_out. The certificate itself (|delta_n|^2 < thr_b with
thr_b from `cell`'s Gershgorin bounds) is evaluated on the host - it is a
validity guard, not part of the loss dataflow. If any atom fails it, the
host falls back to an exact numpy evaluation of the full 27-image argmin.

Sharding: data-parallel over structures. Core c gets structures
[c*2048, (c+1)*2048) == atoms [c*131072, (c+1)*131072). The only
cross-atom reduction (per-structure scatter-mean) is shard-local; the
final mean is assembled on the host from per-partition partial sums.
"""

import numpy as np
from contextlib import ExitStack

# ---- problem constants (hardcoded per the fixed input spec) ----
B = 16384            # structures
A = 64               # atoms per structure
N = B * A            # total atoms
NCORES = 8
BS = B // NCORES     # structures per core     = 2048
NS = N // NCORES     # atoms per core          = 131072
P = 128              # SBUF partitions
SPP = BS // P        # structures per partition = 16
FC = NS * 3 // P     # f32 elems per partition  = 3072
NCH = 8              # pipeline chunks
SCH = SPP // NCH     # structures per partition per chunk = 2
CW = FC // NCH       # columns per chunk = 384
BUFS = 8
CERT_MARGIN = 4.04

_cache = {}


def _build_nc():
    import concourse.bacc as bacc
    import concourse.tile as tile
    import concourse.mybir as mybir

    f32 = mybir.dt.float32
    Alu = mybir.AluOpType
    Act = mybir.ActivationFunctionType

    nc = bacc.Bacc("TRN2", target_bir_lowering=False, debug=False,
                   num_devices=NCORES)
    xin_d = nc.dram_tensor("xin", [P, 2 * FC], f32, kind="ExternalInput").ap()
    out_d = nc.dram_tensor("out", [P, NCH], f32, kind="ExternalOutput").ap()

    with tile.TileContext(nc) as tc:
        with ExitStack() as ctx:
            singles = ctx.enter_context(tc.tile_pool(name="singles", bufs=1))
            io = ctx.enter_context(tc.tile_pool(name="io", bufs=BUFS))
            work = ctx.enter_context(tc.tile_pool(name="work", bufs=BUFS))
            outs = singles.tile([P, NCH], f32)

            for ch in range(NCH):
                xb = io.tile([P, 2 * CW], f32, tag="xb")
                nc.sync.dma_start(
                    out=xb[:], in_=xin_d[:, ch * 2 * CW:(ch + 1) * 2 * CW])
                xc = xb[:, :CW]
                xtc = xb[:, CW:]
                # planar delta: read [s,a,c]-interleaved, write [s,c,a].
                # Iterate in INPUT order (contiguous 4B reads) and let the
                # writes be strided: the GPSIMD >8B-stride cliff is on the
                # read-side address generator.
                d = work.tile([P, CW], f32, tag="d")
                d_pl = d[:].rearrange("p (s c a) -> p s c a", c=3, a=A)
                # chunk 0's delta on VectorE: it is idle during the ramp and
                # skips the GPSIMD->VectorE hop on the first dependency chain
                deng = nc.vector if ch == 0 else nc.gpsimd
                with tc.high_priority():
                    deng.tensor_sub(
                        d[:].rearrange("p (s c a) -> p s a c", c=3, a=A),
                        xc.rearrange("p (s a c) -> p s a c", a=A, c=3),
                        xtc.rearrange("p (s a c) -> p s a c", a=A, c=3))
                # hop-free VectorE chain: segment sums -> centers -> rt
                segsum = work.tile([P, SCH, 3], f32, tag="segsum")
                nc.vector.tensor_reduce(out=segsum[:], in_=d_pl,
                                        axis=mybir.AxisListType.X, op=Alu.add)
                csc = work.tile([P, SCH, 3], f32, tag="csc")
                nc.vector.tensor_scalar_mul(csc[:], segsum[:], 1.0 / A)
                rt = work.tile([P, CW], f32, tag="rt")
                rt_pl = rt[:].rearrange("p (s c a) -> p s c a", c=3, a=A)
                nc.vector.tensor_sub(rt_pl,
                                     csc[:].broadcast_to([P, SCH, 3, A]), d_pl)
                # sum |rt| via ScalarE Abs + accumulate
                scrap = work.tile([P, CW], f32, tag="scrap")
                nc.scalar.activation(out=scrap[:], in_=rt[:], func=Act.Abs,
                                     accum_out=outs[:, ch:ch + 1])

            nc.sync.dma_start(out=out_d, in_=outs[:])
    nc.compile()
    return nc


def _get_nc():
    if "nc" not in _cache:
        _cache["nc"] = _build_nc()
    return _cache["nc"]


def _host_thr(cell):
    """Per-structure certificate threshold, from structure-level scalars.

    thr_b = gersh_lo(G_b) / (CERT_MARGIN * rowmax(|G_b|)), G = M^T M.
    Degenerate structures (gersh_lo <= 0, zero rows, NaN) get -inf-ish so
    they always trip the certificate and force the exact fallback.
    """
    c64 = cell.astype(np.float64)
    G = np.einsum("bki,bkj->bij", c64, c64)
    absG = np.abs(G)
    diag = np.diagonal(G, axis1=1, axis2=2)
    rowsum = absG.sum(2)
    gersh_lo = (2.0 * diag - rowsum).min(1)
    rowmax = rowsum.max(1)
    with np.errstate(divide="ignore", invalid="ignore"):
        thr = gersh_lo / (CERT_MARGIN * rowmax)
    bad = ~np.isfinite(thr) | (gersh_lo <= 0) | (rowmax <= 0)
    return np.where(bad, -3.0e38, thr)


# ---------------- exact host fallback (never hit for sane inputs) -------
_OFFSETS = np.array([[a, b, c] for a in (-1, 0, 1) for b in (-1, 0, 1)
                     for c in (-1, 0, 1)], dtype=np.float32)


def _reference_numpy(cell, x, x_tilde, num_atoms):
    cell = np.asarray(cell, np.float32)
    x = np.asarray(x, np.float32)
    x_tilde = np.asarray(x_tilde, np.float32)
    num_atoms = np.asarray(num_atoms)
    n = x.shape[0]
    bounds = np.cumsum(num_atoms)
    batch = np.searchsorted(bounds, np.arange(n), side="right")
    d_all = np.empty_like(x)
    for lo in range(0, n, 131072):
        hi = min(lo + 131072, n)
        cb = cell[batch[lo:hi]]                                   # [m,3,3]
        euc_xt = np.einsum("nij,nj->ni", cb, x_tilde[lo:hi])
        frac = x[lo:hi, None, :] + _OFFSETS[None, :, :]           # [m,27,3]
        euc_x = np.einsum("nij,noj->noi", cb, frac)
        dist = np.linalg.norm(euc_xt[:, None, :] - euc_x, axis=2)
        mi = np.argmin(dist, axis=1)
        d_all[lo:hi] = (x[lo:hi] + _OFFSETS[mi]) - x_tilde[lo:hi]
    sums = np.zeros((num_atoms.shape[0], 3), np.float32)
    np.add.at(sums, batch, d_all)
    center = sums / num_atoms.astype(np.float32)[:, None]
    tot = np.abs(d_all - center[batch]).sum(dtype=np.float64)
    return np.float32(tot / d_all.size)


def _make_in_maps(x, x_tilde):
    in_maps = []
    for c in range(NCORES):
        xr = np.ascontiguousarray(x[c * NS:(c + 1) * NS]).reshape(P, NCH, CW)
        xtr = np.ascontiguousarray(
            x_tilde[c * NS:(c + 1) * NS]).reshape(P, NCH, CW)
        xin = np.concatenate([xr, xtr], axis=2).reshape(P, 2 * FC)
        in_maps.append({"xin": np.ascontiguousarray(xin)})
    return in_maps


def _run_device(x, x_tilde, trace=False):
    from concourse.bass_utils import run_bass_kernel_spmd
    nc = _get_nc()
    return run_bass_kernel_spmd(nc, _make_in_maps(x, x_tilde),
                                core_ids=list(range(NCORES)), trace=trace)


def kernel(cell, x, x_tilde, num_atoms):
    cell = np.asarray(cell)
    x = np.asarray(x)
    x_tilde = np.asarray(x_tilde)
    num_atoms = np.asarray(num_atoms)

    shapes_ok = (cell.shape == (B, 3, 3) and x.shape == (N, 3)
                 and x_tilde.shape == (N, 3) and num_atoms.shape == (B,)
                 and np.all(num_atoms == A))
    if not shapes_ok:
        return _reference_numpy(cell, x, x_tilde, num_atoms)

    res = _run_device(np.asarray(x, np.float32),
                      np.asarray(x_tilde, np.float32))
    # certificate (host, exact l2 form): |delta_n|^2 < thr_b for every atom
    thr = _host_thr(np.asarray(cell, np.float32))          # [B] float64
    delta = x.astype(np.float32) - x_tilde.astype(np.float32)
    d2 = (delta.astype(np.float64) ** 2).sum(1).reshape(B, A)
    if not (np.isfinite(d2).all() and (d2.max(1) < thr).all()):
        # some atom might prefer a non-zero periodic image: exact fallback
        return _reference_numpy(cell, x, x_tilde, num_atoms)
    total = 0.0
    for c in range(NCORES):
        psum = res.results[c]["out"]
        if not np.isfinite(psum).all():
            return _reference_numpy(cell, x, x_tilde, num_atoms)
        total += psum.sum(dtype=np.float64)
    return np.float32(total / (3.0 * N))


# revision 15
# speedup vs baseline: 1.0114x; 1.0068x over previous
"""Trainium2 Bass kernel for nn_MinDistanceLoss (min-image distance loss).

Math (per atom n in structure b, with M = cell[b], G = M^T M):
    delta_n = x_n - x_tilde_n
    off*_n  = argmin_{off in {-1,0,1}^3} ||M (delta_n + off)||^2
    d_n     = delta_n + off*_n
    center_b = mean_{n in b} d_n
    loss    = mean |d_n - center_b|        (mean over all atoms and 3 comps)

Key optimization: for every atom that satisfies the *certificate*

    CERT_MARGIN * lambda_max_bound(G_b) * |delta_n|^2 < lambda_min_bound(G_b)

(Gershgorin bounds: lambda_max <= max_i sum_j |G_ij|,
 lambda_min >= min_i (G_ii - sum_{j!=i} |G_ij|); CERT_MARGIN = 4.04 > 4),
off* = 0 is provably the unique argmin: ||M off|| >= 2.01 ||M delta|| for
any off != 0 implies ||M(delta+off)|| >= 1.005 ||M delta||, a margin that
also dominates fp32 rounding in the reference computation. For the
physical regime this model targets (cells ~ 4*I Angstrom, |delta| ~ 0.05
fractional) the certificate holds for every atom with >2x margin.

The device computes, per core shard (struct-aligned, planar delta layout):
delta (GPSIMD), per-structure segment sums -> centers -> rt = center-delta
(a hop-free VectorE chain), and the |rt| partition sums via the ScalarE Abs
activation's accum_out. The certificate itself (|delta_n|^2 < thr_b with
thr_b from `cell`'s Gershgorin bounds) is evaluated on the host - it is a
validity guard, not part of the loss dataflow. If any atom fails it, the
host falls back to an exact numpy evaluation of the full 27-image argmin.

Sharding: data-parallel over structures. Core c gets structures
[c*2048, (c+1)*2048) == atoms [c*131072, (c+1)*131072). The only
cross-atom reduction (per-structure scatter-mean) is shard-local; the
final mean is assembled on the host from per-partition partial sums.
"""

import numpy as np
from contextlib import ExitStack

# ---- problem constants (hardcoded per the fixed input spec) ----
B = 16384            # structures
A = 64               # atoms per structure
N = B * A            # total atoms
NCORES = 8
BS = B // NCORES     # structures per core     = 2048
NS = N // NCORES     # atoms per core          = 131072
P = 128              # SBUF partitions
SPP = BS // P        # structures per partition = 16
FC = NS * 3 // P     # f32 elems per partition  = 3072
NCH = 8              # pipeline chunks
SCH = SPP // NCH     # structures per partition per chunk = 2
CW = FC // NCH       # columns per chunk = 384
BUFS = 8
CERT_MARGIN = 4.04

_cache = {}


def _build_nc():
    import concourse.bacc as bacc
    import concourse.tile as tile
    import concourse.mybir as mybir

    f32 = mybir.dt.float32
    Alu = mybir.AluOpType
    Act = mybir.ActivationFunctionType

    nc = bacc.Bacc("TRN2", target_bir_lowering=False, debug=False,
                   num_devices=NCORES)
    xin_d = nc.dram_tensor("xin", [P, 2 * FC], f32, kind="ExternalInput").ap()
    out_d = nc.dram_tensor("out", [P, NCH], f32, kind="ExternalOutput").ap()

    with tile.TileContext(nc) as tc:
        with ExitStack() as ctx:
            singles = ctx.enter_context(tc.tile_pool(name="singles", bufs=1))
            io = ctx.enter_context(tc.tile_pool(name="io", bufs=BUFS))
            work = ctx.enter_context(tc.tile_pool(name="work", bufs=BUFS))
            outs = singles.tile([P, NCH], f32)

            for ch in range(NCH):
                xb = io.tile([P, 2 * CW], f32, tag="xb")
                nc.sync.dma_start(
                    out=xb[:], in_=xin_d[:, ch * 2 * CW:(ch + 1) * 2 * CW])
                xc = xb[:, :CW]
                xtc = xb[:, CW:]
                # planar delta: read [s,a,c]-interleaved, write [s,c,a].
                # Iterate in INPUT order (contiguous 4B reads) and let the
                # writes be strided: the GPSIMD >8B-stride cliff is on the
                # read-side address generator.
                d = work.tile([P, CW], f32, tag="d")
                d_pl = d[:].rearrange("p (s c a) -> p s c a", c=3, a=A)
                # chunk 0's delta on VectorE: it is idle during the ramp and
                # skips the GPSIMD->VectorE hop on the first dependency chain
                deng = nc.vector if ch == 0 else nc.gpsimd
                with tc.high_priority():
                    deng.tensor_sub(
                        d[:].rearrange("p (s c a) -> p s a c", c=3, a=A),
                        xc.rearrange("p (s a c) -> p s a c", a=A, c=3),
                        xtc.rearrange("p (s a c) -> p s a c", a=A, c=3))
                # hop-free VectorE chain: segment sums -> centers -> rt
                segsum = work.tile([P, SCH, 3], f32, tag="segsum")
                nc.vector.tensor_reduce(out=segsum[:], in_=d_pl,
                                        axis=mybir.AxisListType.X, op=Alu.add)
                csc = work.tile([P, SCH, 3], f32, tag="csc")
                nc.vector.tensor_scalar_mul(csc[:], segsum[:], 1.0 / A)
                rt = work.tile([P, CW], f32, tag="rt")
                rt_pl = rt[:].rearrange("p (s c a) -> p s c a", c=3, a=A)
                nc.vector.tensor_sub(rt_pl,
                                     csc[:].broadcast_to([P, SCH, 3, A]), d_pl)
                if ch == NCH - 1:
                    # last chunk: |rt| sum stays on VectorE (same engine as
                    # rt -> hop-free, shorter drain-down)
                    nc.vector.tensor_reduce(out=outs[:, ch:ch + 1], in_=rt[:],
                                            axis=mybir.AxisListType.X,
                                            op=Alu.add,
                                            apply_absolute_value=True)
                else:
                    # sum |rt| via ScalarE Abs + accumulate (offloads VectorE)
                    scrap = work.tile([P, CW], f32, tag="scrap")
                    nc.scalar.activation(out=scrap[:], in_=rt[:], func=Act.Abs,
                                         accum_out=outs[:, ch:ch + 1])

            nc.sync.dma_start(out=out_d, in_=outs[:])
    nc.compile()
    return nc


def _get_nc():
    if "nc" not in _cache:
        _cache["nc"] = _build_nc()
    return _cache["nc"]


def _host_thr(cell):
    """Per-structure certificate threshold, from structure-level scalars.

    thr_b = gersh_lo(G_b) / (CERT_MARGIN * rowmax(|G_b|)), G = M^T M.
    Degenerate structures (gersh_lo <= 0, zero rows, NaN) get -inf-ish so
    they always trip the certificate and force the exact fallback.
    """
    c64 = cell.astype(np.float64)
    G = np.einsum("bki,bkj->bij", c64, c64)
    absG = np.abs(G)
    diag = np.diagonal(G, axis1=1, axis2=2)
    rowsum = absG.sum(2)
    gersh_lo = (2.0 * diag - rowsum).min(1)
    rowmax = rowsum.max(1)
    with np.errstate(divide="ignore", invalid="ignore"):
        thr = gersh_lo / (CERT_MARGIN * rowmax)
    bad = ~np.isfinite(thr) | (gersh_lo <= 0) | (rowmax <= 0)
    return np.where(bad, -3.0e38, thr)


# ---------------- exact host fallback (never hit for sane inputs) -------
_OFFSETS = np.array([[a, b, c] for a in (-1, 0, 1) for b in (-1, 0, 1)
                     for c in (-1, 0, 1)], dtype=np.float32)


def _reference_numpy(cell, x, x_tilde, num_atoms):
    cell = np.asarray(cell, np.float32)
    x = np.asarray(x, np.float32)
    x_tilde = np.asarray(x_tilde, np.float32)
    num_atoms = np.asarray(num_atoms)
    n = x.shape[0]
    bounds = np.cumsum(num_atoms)
    batch = np.searchsorted(bounds, np.arange(n), side="right")
    d_all = np.empty_like(x)
    for lo in range(0, n, 131072):
        hi = min(lo + 131072, n)
        cb = cell[batch[lo:hi]]                                   # [m,3,3]
        euc_xt = np.einsum("nij,nj->ni", cb, x_tilde[lo:hi])
        frac = x[lo:hi, None, :] + _OFFSETS[None, :, :]           # [m,27,3]
        euc_x = np.einsum("nij,noj->noi", cb, frac)
        dist = np.linalg.norm(euc_xt[:, None, :] - euc_x, axis=2)
        mi = np.argmin(dist, axis=1)
        d_all[lo:hi] = (x[lo:hi] + _OFFSETS[mi]) - x_tilde[lo:hi]
    sums = np.zeros((num_atoms.shape[0], 3), np.float32)
    np.add.at(sums, batch, d_all)
    center = sums / num_atoms.astype(np.float32)[:, None]
    tot = np.abs(d_all - center[batch]).sum(dtype=np.float64)
    return np.float32(tot / d_all.size)


def _make_in_maps(x, x_tilde):
    in_maps = []
    for c in range(NCORES):
        xr = np.ascontiguousarray(x[c * NS:(c + 1) * NS]).reshape(P, NCH, CW)
        xtr = np.ascontiguousarray(
            x_tilde[c * NS:(c + 1) * NS]).reshape(P, NCH, CW)
        xin = np.concatenate([xr, xtr], axis=2).reshape(P, 2 * FC)
        in_maps.append({"xin": np.ascontiguousarray(xin)})
    return in_maps


def _run_device(x, x_tilde, trace=False):
    from concourse.bass_utils import run_bass_kernel_spmd
    nc = _get_nc()
    return run_bass_kernel_spmd(nc, _make_in_maps(x, x_tilde),
                                core_ids=list(range(NCORES)), trace=trace)


def kernel(cell, x, x_tilde, num_atoms):
    cell = np.asarray(cell)
    x = np.asarray(x)
    x_tilde = np.asarray(x_tilde)
    num_atoms = np.asarray(num_atoms)

    shapes_ok = (cell.shape == (B, 3, 3) and x.shape == (N, 3)
                 and x_tilde.shape == (N, 3) and num_atoms.shape == (B,)
                 and np.all(num_atoms == A))
    if not shapes_ok:
        return _reference_numpy(cell, x, x_tilde, num_atoms)

    res = _run_device(np.asarray(x, np.float32),
                      np.asarray(x_tilde, np.float32))
    # certificate (host, exact l2 form): |delta_n|^2 < thr_b for every atom
    thr = _host_thr(np.asarray(cell, np.float32))          # [B] float64
    delta = x.astype(np.float32) - x_tilde.astype(np.float32)
    d2 = (delta.astype(np.float64) ** 2).sum(1).reshape(B, A)
    if not (np.isfinite(d2).all() and (d2.max(1) < thr).all()):
        # some atom might prefer a non-zero periodic image: exact fallback
        return _reference_numpy(cell, x, x_tilde, num_atoms)
    total = 0.0
    for c in range(NCORES):
        psum = res.results[c]["out"]
        if not np.isfinite(psum).all():
            return _reference_numpy(cell, x, x_tilde, num_atoms)
        total += psum.sum(dtype=np.float64)
    return np.float32(total / (3.0 * N))


# revision 16
# speedup vs baseline: 1.0402x; 1.0285x over previous
"""Trainium2 Bass kernel for nn_MinDistanceLoss (min-image distance loss).

Math (per atom n in structure b, with M = cell[b], G = M^T M):
    delta_n = x_n - x_tilde_n
    off*_n  = argmin_{off in {-1,0,1}^3} ||M (delta_n + off)||^2
    d_n     = delta_n + off*_n
    center_b = mean_{n in b} d_n
    loss    = mean |d_n - center_b|        (mean over all atoms and 3 comps)

Key optimization: for every atom that satisfies the *certificate*

    CERT_MARGIN * lambda_max_bound(G_b) * |delta_n|^2 < lambda_min_bound(G_b)

(Gershgorin bounds: lambda_max <= max_i sum_j |G_ij|,
 lambda_min >= min_i (G_ii - sum_{j!=i} |G_ij|); CERT_MARGIN = 4.04 > 4),
off* = 0 is provably the unique argmin: ||M off|| >= 2.01 ||M delta|| for
any off != 0 implies ||M(delta+off)|| >= 1.005 ||M delta||, a margin that
also dominates fp32 rounding in the reference computation. For the
physical regime this model targets (cells ~ 4*I Angstrom, |delta| ~ 0.05
fractional) the certificate holds for every atom with >2x margin.

The device computes, per core shard (struct-aligned, planar delta layout):
delta (GPSIMD), per-structure segment sums -> centers -> rt = center-delta
(a hop-free VectorE chain), and the |rt| partition sums via the ScalarE Abs
activation's accum_out. The certificate itself (|delta_n|^2 < thr_b with
thr_b from `cell`'s Gershgorin bounds) is evaluated on the host - it is a
validity guard, not part of the loss dataflow. If any atom fails it, the
host falls back to an exact numpy evaluation of the full 27-image argmin.

Sharding: data-parallel over structures. Core c gets structures
[c*2048, (c+1)*2048) == atoms [c*131072, (c+1)*131072). The only
cross-atom reduction (per-structure scatter-mean) is shard-local; the
final mean is assembled on the host from per-partition partial sums.
"""

import numpy as np
from contextlib import ExitStack

# ---- problem constants (hardcoded per the fixed input spec) ----
B = 16384            # structures
A = 64               # atoms per structure
N = B * A            # total atoms
NCORES = 8
BS = B // NCORES     # structures per core     = 2048
NS = N // NCORES     # atoms per core          = 131072
P = 128              # SBUF partitions
SPP = BS // P        # structures per partition = 16
FC = NS * 3 // P     # f32 elems per partition  = 3072
NCH = 8              # pipeline chunks
SCH = SPP // NCH     # structures per partition per chunk = 2
CW = FC // NCH       # columns per chunk = 384
BUFS = 8
CERT_MARGIN = 4.04

_cache = {}


def _build_nc():
    import concourse.bacc as bacc
    import concourse.tile as tile
    import concourse.mybir as mybir

    f32 = mybir.dt.float32
    Alu = mybir.AluOpType
    Act = mybir.ActivationFunctionType

    nc = bacc.Bacc("TRN2", target_bir_lowering=False, debug=False,
                   num_devices=NCORES)
    xin_d = nc.dram_tensor("xin", [P, 2 * FC], f32, kind="ExternalInput").ap()
    out_d = nc.dram_tensor("out", [P, NCH], f32, kind="ExternalOutput").ap()

    with tile.TileContext(nc) as tc:
        with ExitStack() as ctx:
            singles = ctx.enter_context(tc.tile_pool(name="singles", bufs=1))
            io = ctx.enter_context(tc.tile_pool(name="io", bufs=BUFS))
            work = ctx.enter_context(tc.tile_pool(name="work", bufs=BUFS))
            outs = singles.tile([P, NCH], f32)

            for ch in range(NCH):
                xb = io.tile([P, 2 * CW], f32, tag="xb")
                nc.sync.dma_start(
                    out=xb[:], in_=xin_d[:, ch * 2 * CW:(ch + 1) * 2 * CW])
                xc = xb[:, :CW]
                xtc = xb[:, CW:]
                # planar delta: read [s,a,c]-interleaved, write [s,c,a].
                # Iterate in INPUT order (contiguous 4B reads) and let the
                # writes be strided: the GPSIMD >8B-stride cliff is on the
                # read-side address generator.
                d = work.tile([P, CW], f32, tag="d")
                d_pl = d[:].rearrange("p (s c a) -> p s c a", c=3, a=A)
                # chunk 0's delta on VectorE: it is idle during the ramp and
                # skips the GPSIMD->VectorE hop on the first dependency chain
                deng = nc.vector if ch == 0 else nc.gpsimd
                with tc.high_priority():
                    deng.tensor_sub(
                        d[:].rearrange("p (s c a) -> p s a c", c=3, a=A),
                        xc.rearrange("p (s a c) -> p s a c", a=A, c=3),
                        xtc.rearrange("p (s a c) -> p s a c", a=A, c=3))
                # hop-free VectorE chain: segment sums -> centers -> rt
                segsum = work.tile([P, SCH, 3], f32, tag="segsum")
                nc.vector.tensor_reduce(out=segsum[:], in_=d_pl,
                                        axis=mybir.AxisListType.X, op=Alu.add)
                csc = work.tile([P, SCH, 3], f32, tag="csc")
                nc.vector.tensor_scalar_mul(csc[:], segsum[:], 1.0 / A)
                rt = work.tile([P, CW], f32, tag="rt")
                rt_pl = rt[:].rearrange("p (s c a) -> p s c a", c=3, a=A)
                # chunk NCH-2's rt on GPSIMD (idle after its deltas): frees
                # VectorE to start the last chunk's hop-free tail chain sooner
                reng = nc.gpsimd if ch == NCH - 2 else nc.vector
                reng.tensor_sub(rt_pl,
                                csc[:].broadcast_to([P, SCH, 3, A]), d_pl)
                if ch == NCH - 1:
                    # last chunk: |rt| sum stays on VectorE (same engine as
                    # rt -> hop-free, shorter drain-down)
                    nc.vector.tensor_reduce(out=outs[:, ch:ch + 1], in_=rt[:],
                                            axis=mybir.AxisListType.X,
                                            op=Alu.add,
                                            apply_absolute_value=True)
                else:
                    # sum |rt| via ScalarE Abs + accumulate (offloads VectorE)
                    scrap = work.tile([P, CW], f32, tag="scrap")
                    nc.scalar.activation(out=scrap[:], in_=rt[:], func=Act.Abs,
                                         accum_out=outs[:, ch:ch + 1])

            nc.sync.dma_start(out=out_d, in_=outs[:])
    nc.compile()
    return nc


def _get_nc():
    if "nc" not in _cache:
        _cache["nc"] = _build_nc()
    return _cache["nc"]


def _host_thr(cell):
    """Per-structure certificate threshold, from structure-level scalars.

    thr_b = gersh_lo(G_b) / (CERT_MARGIN * rowmax(|G_b|)), G = M^T M.
    Degenerate structures (gersh_lo <= 0, zero rows, NaN) get -inf-ish so
    they always trip the certificate and force the exact fallback.
    """
    c64 = cell.astype(np.float64)
    G = np.einsum("bki,bkj->bij", c64, c64)
    absG = np.abs(G)
    diag = np.diagonal(G, axis1=1, axis2=2)
    rowsum = absG.sum(2)
    gersh_lo = (2.0 * diag - rowsum).min(1)
    rowmax = rowsum.max(1)
    with np.errstate(divide="ignore", invalid="ignore"):
        thr = gersh_lo / (CERT_MARGIN * rowmax)
    bad = ~np.isfinite(thr) | (gersh_lo <= 0) | (rowmax <= 0)
    return np.where(bad, -3.0e38, thr)


# ---------------- exact host fallback (never hit for sane inputs) -------
_OFFSETS = np.array([[a, b, c] for a in (-1, 0, 1) for b in (-1, 0, 1)
                     for c in (-1, 0, 1)], dtype=np.float32)


def _reference_numpy(cell, x, x_tilde, num_atoms):
    cell = np.asarray(cell, np.float32)
    x = np.asarray(x, np.float32)
    x_tilde = np.asarray(x_tilde, np.float32)
    num_atoms = np.asarray(num_atoms)
    n = x.shape[0]
    bounds = np.cumsum(num_atoms)
    batch = np.searchsorted(bounds, np.arange(n), side="right")
    d_all = np.empty_like(x)
    for lo in range(0, n, 131072):
        hi = min(lo + 131072, n)
        cb = cell[batch[lo:hi]]                                   # [m,3,3]
        euc_xt = np.einsum("nij,nj->ni", cb, x_tilde[lo:hi])
        frac = x[lo:hi, None, :] + _OFFSETS[None, :, :]           # [m,27,3]
        euc_x = np.einsum("nij,noj->noi", cb, frac)
        dist = np.linalg.norm(euc_xt[:, None, :] - euc_x, axis=2)
        mi = np.argmin(dist, axis=1)
        d_all[lo:hi] = (x[lo:hi] + _OFFSETS[mi]) - x_tilde[lo:hi]
    sums = np.zeros((num_atoms.shape[0], 3), np.float32)
    np.add.at(sums, batch, d_all)
    center = sums / num_atoms.astype(np.float32)[:, None]
    tot = np.abs(d_all - center[batch]).sum(dtype=np.float64)
    return np.float32(tot / d_all.size)


def _make_in_maps(x, x_tilde):
    in_maps = []
    for c in range(NCORES):
        xr = np.ascontiguousarray(x[c * NS:(c + 1) * NS]).reshape(P, NCH, CW)
        xtr = np.ascontiguousarray(
            x_tilde[c * NS:(c + 1) * NS]).reshape(P, NCH, CW)
        xin = np.concatenate([xr, xtr], axis=2).reshape(P, 2 * FC)
        in_maps.append({"xin": np.ascontiguousarray(xin)})
    return in_maps


def _run_device(x, x_tilde, trace=False):
    from concourse.bass_utils import run_bass_kernel_spmd
    nc = _get_nc()
    return run_bass_kernel_spmd(nc, _make_in_maps(x, x_tilde),
                                core_ids=list(range(NCORES)), trace=trace)


def kernel(cell, x, x_tilde, num_atoms):
    cell = np.asarray(cell)
    x = np.asarray(x)
    x_tilde = np.asarray(x_tilde)
    num_atoms = np.asarray(num_atoms)

    shapes_ok = (cell.shape == (B, 3, 3) and x.shape == (N, 3)
                 and x_tilde.shape == (N, 3) and num_atoms.shape == (B,)
                 and np.all(num_atoms == A))
    if not shapes_ok:
        return _reference_numpy(cell, x, x_tilde, num_atoms)

    res = _run_device(np.asarray(x, np.float32),
                      np.asarray(x_tilde, np.float32))
    # certificate (host, exact l2 form): |delta_n|^2 < thr_b for every atom
    thr = _host_thr(np.asarray(cell, np.float32))          # [B] float64
    delta = x.astype(np.float32) - x_tilde.astype(np.float32)
    d2 = (delta.astype(np.float64) ** 2).sum(1).reshape(B, A)
    if not (np.isfinite(d2).all() and (d2.max(1) < thr).all()):
        # some atom might prefer a non-zero periodic image: exact fallback
        return _reference_numpy(cell, x, x_tilde, num_atoms)
    total = 0.0
    for c in range(NCORES):
        psum = res.results[c]["out"]
        if not np.isfinite(psum).all():
            return _reference_numpy(cell, x, x_tilde, num_atoms)
        total += psum.sum(dtype=np.float64)
    return np.float32(total / (3.0 * N))
